# revision 17
# baseline (speedup 1.0000x reference)
"""ConvLSTM2D forward on 8 Trainium2 NeuronCores (v4.3).

Problem: x [8,10,256,256,8], Wx [3,3,8,4], Wh [3,3,1,4], b [4]
         -> h_last [8,256,256,1]  (ConvLSTM, keras gate order i,f,c,o;
         i/f/o hard_sigmoid, candidate+output sigmoid)

Sharding: data-parallel over batch; core b computes batch element b fully
locally (recurrent scan stays on-core, no collectives in forward).

v4.3 (baseline v3 was 213us):
 - x for ALL 10 steps lives in SBUF (123.8KB/partition) and is loaded by
   a handful of multi-step chunk DMAs issued in the prologue.  DRAM
   layout is partition-major so each chunk is one ~12-25KB descriptor
   per partition: the SWDGE Q7 generates only ~9 descriptors/us, which
   made any per-step 102-descriptor load ~11us wall (the v3/v4 PE-starve).
   Chunks split across the gpsimd SWDGE path (16 SDMA engines) and the
   sync HWDGE path (6 engines) which run in parallel.
 - the t>=1 h-halo sections are NOT loaded (windows write them), so the
   recurrent h scatter never waits on a load; halo edge rows come from
   two tiny zero DMAs.
 - gate affine 0.2*z+b+0.5 folded INTO the matmul: weights for i/f/o
   pre-scaled by 0.2, K grows to 103 with a constant-1 rhs row (packed
   in DRAM) carrying the biases via the cg0/kw0 pass.
 - all-bf16 epilogue; deinterleave 32p ops on DVE/ACT only (GpSimd ALU
   is ~6x slower - it only issues DMAs); h scatter via 10 DMAs on
   sync/gpsimd; per-pair output DMAs on the final step.
"""

import numpy as np
import ml_dtypes

import concourse.bacc as bacc
import concourse.bass as bass
import concourse.mybir as mybir
import concourse.tile as tile
from concourse import bass_utils

F32 = mybir.dt.float32
BF16 = mybir.dt.bfloat16
AF = mybir.ActivationFunctionType
OP = mybir.AluOpType

B, T, H, W, CIN = 8, 10, 256, 256, 8
G = 4            # gates i,f,c,o
RT = 32          # output rows per tile (M = G*RT = 128)
TAU = H // RT    # 8 row tiles
HIN = RT + 2     # input rows per tile (with halo)
CPG = 3          # channels per contraction group
NCG = 3          # channel groups (3,3,2+h)
KP = HIN * CPG   # 102 partitions of conv data per rhs tile
KB = KP + 1      # +1 constant-1 bias row
NPAIR = TAU // 2 # 4 tau-pairs (tau, tau+4) -> N=512 matmuls
WP = W + 2       # padded width
SPLIT = 68       # partitions [0:SPLIT) load via SWDGE, [SPLIT:KB) via HWDGE
                 # (0:68 = cc0/cc1 rows: ALL x channels of every cg;
                 #  68:102 = cc2 rows: ch2 (cg0), ch5 (cg1), h (cg2))
# chunk boundaries in t for the prologue loads
CHUNKS = [(0, 1), (1, 2), (2, 3), (3, 5), (5, 7), (7, 9), (9, 10)]


def h_window_segments():
    """(tau, seg_lo, seg_hi, planar_part0, planar_blk) for the h halo windows.

    Window rows for tau: 32*tau-1 .. 32*tau+32 (lr 0..33) at partition 68+lr;
    segments split where the window crosses the planar block boundary.
    """
    out = []
    for tau in range(TAU):
        r0 = tau * RT - 1
        lo = max(0, -r0)
        hi = min(HIN, H - r0)
        s = lo
        while s < hi:
            blk = (r0 + s) // 128
            e = min(hi, (blk + 1) * 128 - r0)
            out.append((tau, s, e, r0 + s - blk * 128, blk))
            s = e
    return out


def pack_inputs(x, Wx, Wh, b):
    """Host-side repack to bf16 device layouts (partition-major x).

    xk[b, cc*34+lr, t, cg, tau, 1+c] = x[b, t, 32*tau-1+lr, c, 3*cg+cc]
    xk[b, 102, t, 0] = 1 (bias row); wb columns carry 0.2-scaled weights
    and the hard-sigmoid biases on the cg0/kw0 pass.
    """
    x = np.asarray(x, dtype=np.float32)
    b = np.asarray(b, dtype=np.float32)
    W9 = np.concatenate([np.asarray(Wx, np.float32),
                         np.asarray(Wh, np.float32)], axis=2)  # [3,3,9,4]
    gscale = np.array([0.2, 0.2, 1.0, 0.2], np.float32)
    gbias = np.array([0.2 * b[0] + 0.5, 0.2 * b[1] + 0.5,
                      b[2], 0.2 * b[3] + 0.5], np.float32)

    xk = np.zeros((B, KB, T, NCG, TAU, WP), dtype=ml_dtypes.bfloat16)
    xb = x.astype(ml_dtypes.bfloat16)
    for tau in range(TAU):
        r0 = tau * RT - 1
        lo = max(0, -r0)
        hi = min(HIN, H - r0)
        for cg in range(NCG):
            for cc in range(CPG):
                ch = cg * CPG + cc
                if ch >= CIN:
                    continue  # h channel: written on device
                xk[:, cc * HIN + lo:cc * HIN + hi, :, cg, tau, 1:W + 1] = \
                    np.moveaxis(xb[:, :, r0 + lo:r0 + hi, :, ch], 1, 2)
    xk[:, KP, :, 0] = 1.0  # bias row

    wb = np.zeros((KB, NCG * 3, G * RT), dtype=np.float32)
    r = np.arange(RT)
    for cg in range(NCG):
        for cc in range(CPG):
            ch = cg * CPG + cc
            for kh in range(3):
                for kw in range(3):
                    for g in range(G):
                        wb[cc * HIN + r + kh, 3 * cg + kw, g * RT + r] = \
                            W9[kh, kw, ch, g] * gscale[g]
    for g in range(G):
        wb[KP, 0, g * RT + r] = gbias[g]
    return xk, wb.astype(ml_dtypes.bfloat16)


def build_program(Tn):
    nc = bacc.Bacc("TRN2", target_bir_lowering=False, debug=False)
    xk_d = nc.dram_tensor("xk", [KB, Tn, NCG, TAU, WP], BF16, kind="ExternalInput")
    zd_d = nc.dram_tensor("zd", [2, Tn, WP], BF16, kind="ExternalInput")
    wb_d = nc.dram_tensor("wb", [KB, NCG * 3, G * RT], BF16, kind="ExternalInput")
    out_d = nc.dram_tensor("out", [H, W], F32, kind="ExternalOutput")

    segs = sorted(h_window_segments(), key=lambda g: (g[0] % NPAIR, g[0]))

    with tile.TileContext(nc) as tc:
        with tc.tile_pool(name="wpool", bufs=1) as wpool, \
             tc.tile_pool(name="gpool", bufs=2) as gpool, \
             tc.tile_pool(name="state", bufs=1) as state, \
             tc.tile_pool(name="zpsum", bufs=2, space="PSUM") as zpsum:

            # --- static weights / state / resident x ---
            wt = wpool.tile([KB, NCG * 3, G * RT], BF16, tag="wt", name="wt")
            nc.scalar.dma_start(out=wt, in_=wb_d[:])

            xt = state.tile([KB, Tn, NCG, TAU, WP], BF16, tag="xt", name="xt")
            for (t0, t1) in CHUNKS:
                # SWDGE (gpsimd, 16 SDMA engines): low partitions, full cgs
                nc.gpsimd.dma_start(out=xt[0:SPLIT, t0:t1],
                                    in_=xk_d[0:SPLIT, t0:t1])
                # HWDGE (sync, 6 engines): high partitions; t0 chunk carries
                # the full h-section zeros, later chunks skip cg2 (windows
                # write it -> no WAW between loads and the recurrence)
                if t0 == 0:
                    nc.sync.dma_start(out=xt[SPLIT:KB, t0:t1],
                                      in_=xk_d[SPLIT:KB, t0:t1])
                else:
                    nc.sync.dma_start(out=xt[SPLIT:KP, t0:t1, 0:2],
                                      in_=xk_d[SPLIT:KP, t0:t1, 0:2])
                    # bias row incl. cg2 zeros (garbage*0 would make NaN)
                    nc.sync.dma_start(out=xt[KP:KB, t0:t1],
                                      in_=xk_d[KP:KB, t0:t1])
            # halo edge rows of the h sections for t>=1 (never window-written)
            nc.scalar.dma_start(out=xt[68:69, 1:Tn, 2, 0], in_=zd_d[0, 1:Tn])
            nc.scalar.dma_start(out=xt[101:102, 1:Tn, 2, TAU - 1],
                                in_=zd_d[1, 1:Tn])

            cbuf = state.tile([128, 2, W], BF16, tag="cbuf", name="cbuf")
            nc.vector.memset(cbuf, 0.0)
            hbuf = state.tile([128, 2, WP], BF16, tag="hbuf", name="hbuf")
            nc.vector.memset(hbuf, 0.0)

            for t in range(Tn):
                # --- matmuls: 9 accumulating passes x 4 tau-pairs, N=512 ---
                xv = xt[:, t].rearrange("p cg (b q) c -> p cg b q c", b=2)
                zt = [zpsum.tile([G * RT, 2, W], F32, tag=f"z{q}", name=f"z{q}")
                      for q in range(NPAIR)]
                gi = gpool.tile([128, 2, W], BF16, tag="gi", name="gi")
                gf = gpool.tile([128, 2, W], BF16, tag="gf", name="gf")
                go = gpool.tile([128, 2, W], BF16, tag="go", name="go")
                sc = gpool.tile([128, 2, W], BF16, tag="sc", name="sc")
                s2 = gpool.tile([128, 2, W], BF16, tag="s2", name="s2")
                t2 = gpool.tile([128, 2, W], BF16, tag="t2", name="t2")

                for cg in range(2):
                    for kw in range(3):
                        for q in range(NPAIR):
                            nc.tensor.matmul(
                                zt[q], wt[:, 3 * cg + kw],
                                xv[:, cg, :, q, kw:kw + W],
                                start=(cg == 0 and kw == 0), stop=False)

                for q in range(NPAIR):
                    for kw in range(3):
                        nc.tensor.matmul(
                            zt[q], wt[:, 6 + kw],
                            xv[:, 2, :, q, kw:kw + W],
                            start=False, stop=(kw == 2))
                    # per-pair PSUM deinterleave (32p forced by PSUM gate
                    # groups); partitions g*32+r -> planar q*32+r, bf16 out.
                    # PE already applied 0.2*z + bias via the constant row.
                    sl = slice(q * RT, (q + 1) * RT)
                    zi, zf, zc, zo = (zt[q][g_ * RT:(g_ + 1) * RT]
                                      for g_ in range(4))
                    nc.scalar.activation(out=sc[sl], in_=zc, func=AF.Sigmoid,
                                         bias=0.0, scale=1.0)
                    nc.scalar.activation(out=go[sl], in_=zo, func=AF.Relu,
                                         bias=0.0, scale=1.0)
                    nc.vector.tensor_scalar(
                        out=gi[sl], in0=zi, scalar1=0.0, scalar2=1.0,
                        op0=OP.max, op1=OP.min)
                    nc.vector.tensor_scalar(
                        out=gf[sl], in0=zf, scalar1=0.0, scalar2=1.0,
                        op0=OP.max, op1=OP.min)

                # --- full-width (128p) bf16 gate math ---
                nc.vector.tensor_scalar(out=go, in0=go, scalar1=1.0,
                                        scalar2=0.0, op0=OP.min, op1=OP.max)
                nc.vector.tensor_tensor(out=t2, in0=gf, in1=cbuf, op=OP.mult)
                nc.vector.tensor_tensor(out=gi, in0=gi, in1=sc, op=OP.mult)
                nc.vector.tensor_tensor(out=cbuf, in0=gi, in1=t2, op=OP.add)
                nc.scalar.activation(out=s2, in_=cbuf, func=AF.Sigmoid,
                                     bias=0.0, scale=1.0)

                if t + 1 < Tn:
                    nc.vector.tensor_tensor(out=hbuf[:, :, 1:W + 1], in0=go,
                                            in1=s2, op=OP.mult)
                    # h scatter into the next step's halo windows: engine ops
                    # can only shift partitions by multiples of 32 -> DMA.
                    engs = (nc.sync, nc.gpsimd)
                    for n, (tau, s, e, p0, blk) in enumerate(segs):
                        engs[n % 2].dma_start(
                            out=xt[68 + s:68 + e, t + 1, 2, tau, :],
                            in_=hbuf[p0:p0 + (e - s), blk, :])
                else:
                    # final step: h = go*s2 per pair -> DRAM
                    hf = gpool.tile([128, 2, W], F32, tag="hf", name="hf")
                    ov = out_d.rearrange("(b p) w -> p b w", p=128)
                    oeng = (nc.sync, nc.gpsimd)
                    for q in range(NPAIR):
                        sl = slice(q * RT, (q + 1) * RT)
                        nc.vector.tensor_tensor(
                            out=hf[sl], in0=go[sl], in1=s2[sl], op=OP.mult)
                        oeng[q % 2].dma_start(out=ov[sl], in_=hf[sl])
    nc.compile()
    return nc


_CACHE = {}


def _get_program(Tn):
    if Tn not in _CACHE:
        _CACHE[Tn] = build_program(Tn)
    return _CACHE[Tn]


def kernel(x, Wx, Wh, b, _run_opts=None):
    x = np.asarray(x, dtype=np.float32)
    Bn, Tn = x.shape[0], x.shape[1]
    xk, wb = pack_inputs(x, Wx, Wh, b)
    nc = _get_program(Tn)
    zd = np.zeros((2, Tn, WP), dtype=ml_dtypes.bfloat16)
    in_maps = [{"xk": np.ascontiguousarray(xk[bi]), "wb": wb, "zd": zd}
               for bi in range(Bn)]
    res = bass_utils.run_bass_kernel_spmd(
        nc, in_maps, core_ids=list(range(Bn)), **(_run_opts or {}))
    out = np.stack([res.results[bi]["out"] for bi in range(Bn)], axis=0)
    kernel.last_results = res
    return out[..., None].astype(np.float32)


# revision 20
# speedup vs baseline: 1.3026x; 1.3026x over previous
"""ConvLSTM2D forward on 8 Trainium2 NeuronCores (v4.5).

Problem: x [8,10,256,256,8], Wx [3,3,8,4], Wh [3,3,1,4], b [4]
         -> h_last [8,256,256,1]  (ConvLSTM, keras gate order i,f,c,o;
         i/f/o hard_sigmoid, candidate+output sigmoid)

Sharding: data-parallel over batch; core b computes batch element b fully
locally (recurrent scan stays on-core, no collectives in forward).

v4.5 (baseline v3 was 213us).  Measured platform facts this build is
shaped around: DMA cost ~ 45-170ns/descriptor + ~55ns/KB per engine
stream; the SWDGE (gpsimd) queue generates ~9 descriptors/us but sprays
16 SDMA engines; the sync HWDGE ring has ~6 engines, scalar ~2;
descriptors >16KB shatter into inefficient ~2.2KB packets; GpSimd ALU is
~6x slower than DVE (DMA issue only).

 - x for ALL 10 steps is resident in SBUF (123.8KB/partition,
   partition-major DRAM, one 12.4KB single-packet descriptor per
   partition per step), loaded per-step 2 steps ahead, split:
   gpsimd [0:56] / scalar [56:68] / sync [68:102]-minus-h + bias row.
 - tau dimension stored pair-major (tau_hat = 2q+b, image tau = 4b+q) so
   each h halo window pair (tau, tau+4) - which shares one planar
   partition range - scatters as ONE dma with 1032B descriptors; 4 pair
   DMAs + 2 single-row DMAs per step on the sync ring, issued BEFORE the
   prefetch loads so they never queue behind them.
 - t>=1 h-halo sections are never loaded (the recurrence writes them);
   halo edge rows come from two tiny zero DMAs.
 - gate affine 0.2*z+b+0.5 folded INTO the matmul (0.2-scaled weights,
   K=103 with a DRAM-packed constant-1 bias row applied on cg0/kw0).
 - all-bf16 epilogue; PSUM deinterleave 32p ops on DVE/ACT only.
"""

import numpy as np
import ml_dtypes

import concourse.bacc as bacc
import concourse.bass as bass
import concourse.mybir as mybir
import concourse.tile as tile
from concourse import bass_utils

F32 = mybir.dt.float32
BF16 = mybir.dt.bfloat16
AF = mybir.ActivationFunctionType
OP = mybir.AluOpType

B, T, H, W, CIN = 8, 10, 256, 256, 8
G = 4            # gates i,f,c,o
RT = 32          # output rows per tile (M = G*RT = 128)
TAU = H // RT    # 8 row tiles
HIN = RT + 2     # input rows per tile (with halo)
CPG = 3          # channels per contraction group
NCG = 3          # channel groups (3,3,2+h)
KP = HIN * CPG   # 102 partitions of conv data per rhs tile
KB = KP + 1      # +1 constant-1 bias row
NPAIR = TAU // 2 # 4 tau-pairs (tau=4b+q at tau_hat=2q+b) -> N=512 matmuls
WP = W + 2       # padded width

# h-window pair DMAs: (dst_lo, dst_hi, src_p0, q); dst partitions 68+s,
# src hbuf[p0 + (s - s_lo), blk], tau_hat (2q, 2q+1) <-> blk (0, 1)
WPAIRS = [(69, 102, 0, 0), (68, 102, 31, 1), (68, 102, 63, 2),
          (68, 101, 95, 3)]


def pack_inputs(x, Wx, Wh, b):
    """Host-side repack to bf16 device layouts (partition-major x,
    pair-major tau).

    xk[b, cc*34+lr, t, cg, 2*(tau%4)+tau//4, 1+c] =
        x[b, t, 32*tau-1+lr, c, 3*cg+cc]
    xk[b, 102, t, 0] = 1 (bias row); wb columns carry 0.2-scaled weights
    and the hard-sigmoid biases on the cg0/kw0 pass.
    """
    x = np.asarray(x, dtype=np.float32)
    b = np.asarray(b, dtype=np.float32)
    W9 = np.concatenate([np.asarray(Wx, np.float32),
                         np.asarray(Wh, np.float32)], axis=2)  # [3,3,9,4]
    gscale = np.array([0.2, 0.2, 1.0, 0.2], np.float32)
    gbias = np.array([0.2 * b[0] + 0.5, 0.2 * b[1] + 0.5,
                      b[2], 0.2 * b[3] + 0.5], np.float32)

    xk = np.zeros((B, KB, T, NCG, TAU, WP), dtype=ml_dtypes.bfloat16)
    xb = x.astype(ml_dtypes.bfloat16)
    for tau in range(TAU):
        th = 2 * (tau % NPAIR) + tau // NPAIR  # pair-major storage index
        r0 = tau * RT - 1
        lo = max(0, -r0)
        hi = min(HIN, H - r0)
        for cg in range(NCG):
            for cc in range(CPG):
                ch = cg * CPG + cc
                if ch >= CIN:
                    continue  # h channel: written on device
                xk[:, cc * HIN + lo:cc * HIN + hi, :, cg, th, 1:W + 1] = \
                    np.moveaxis(xb[:, :, r0 + lo:r0 + hi, :, ch], 1, 2)
    xk[:, KP, :, 0] = 1.0  # bias row

    wb = np.zeros((KB, NCG * 3, G * RT), dtype=np.float32)
    r = np.arange(RT)
    for cg in range(NCG):
        for cc in range(CPG):
            ch = cg * CPG + cc
            for kh in range(3):
                for kw in range(3):
                    for g in range(G):
                        wb[cc * HIN + r + kh, 3 * cg + kw, g * RT + r] = \
                            W9[kh, kw, ch, g] * gscale[g]
    for g in range(G):
        wb[KP, 0, g * RT + r] = gbias[g]
    return xk, wb.astype(ml_dtypes.bfloat16)


def build_program(Tn):
    nc = bacc.Bacc("TRN2", target_bir_lowering=False, debug=False)
    xk_d = nc.dram_tensor("xk", [KB, Tn, NCG, TAU, WP], BF16, kind="ExternalInput")
    zd_d = nc.dram_tensor("zd", [2, Tn, WP], BF16, kind="ExternalInput")
    wb_d = nc.dram_tensor("wb", [KB, NCG * 3, G * RT], BF16, kind="ExternalInput")
    out_d = nc.dram_tensor("out", [H, W], F32, kind="ExternalOutput")

    with tile.TileContext(nc) as tc:
        with tc.tile_pool(name="wpool", bufs=1) as wpool, \
             tc.tile_pool(name="gpool", bufs=2) as gpool, \
             tc.tile_pool(name="state", bufs=1) as state, \
             tc.tile_pool(name="zpsum", bufs=2, space="PSUM") as zpsum:

            # --- static weights / state / resident x ---
            wt = wpool.tile([KB, NCG * 3, G * RT], BF16, tag="wt", name="wt")
            nc.sync.dma_start(out=wt, in_=wb_d[:])

            xt = state.tile([KB, Tn, NCG, TAU, WP], BF16, tag="xt", name="xt")

            def load_x(t):
                nc.gpsimd.dma_start(out=xt[0:56, t], in_=xk_d[0:56, t])
                nc.scalar.dma_start(out=xt[56:68, t], in_=xk_d[56:68, t])
                if t == 0:
                    nc.sync.dma_start(out=xt[68:102, 0], in_=xk_d[68:102, 0])
                else:
                    # skip the h section (windows write it; no load WAW)
                    nc.sync.dma_start(out=xt[68:102, t, 0:2],
                                      in_=xk_d[68:102, t, 0:2])
                # bias row incl cg2 zeros (garbage*0 would make NaN)
                nc.sync.dma_start(out=xt[KP:KB, t], in_=xk_d[KP:KB, t])

            load_x(0)
            if Tn > 1:
                load_x(1)
                # halo edge rows of the t>=1 h sections (never window-
                # written): tau0 row -1 (th 0) and tau7 row 256 (th 7)
                nc.scalar.dma_start(out=xt[68:69, 1:Tn, 2, 0],
                                    in_=zd_d[0, 1:Tn])
                nc.scalar.dma_start(out=xt[101:102, 1:Tn, 2, TAU - 1],
                                    in_=zd_d[1, 1:Tn])

            cbuf = state.tile([128, 2, W], BF16, tag="cbuf", name="cbuf")
            nc.vector.memset(cbuf, 0.0)
            hbuf = state.tile([128, 2, WP], BF16, tag="hbuf", name="hbuf")
            nc.vector.memset(hbuf, 0.0)

            for t in range(Tn):
                # --- matmuls: 9 accumulating passes x 4 tau-pairs, N=512 ---
                xv = xt[:, t].rearrange("p cg (q b) c -> p cg q b c", q=NPAIR)
                zt = [zpsum.tile([G * RT, 2, W], F32, tag=f"z{q}", name=f"z{q}")
                      for q in range(NPAIR)]
                gi = gpool.tile([128, 2, W], BF16, tag="gi", name="gi")
                gf = gpool.tile([128, 2, W], BF16, tag="gf", name="gf")
                go = gpool.tile([128, 2, W], BF16, tag="go", name="go")
                sc = gpool.tile([128, 2, W], BF16, tag="sc", name="sc")
                s2 = gpool.tile([128, 2, W], BF16, tag="s2", name="s2")
                t2 = gpool.tile([128, 2, W], BF16, tag="t2", name="t2")

                for cg in range(2):
                    for kw in range(3):
                        for q in range(NPAIR):
                            nc.tensor.matmul(
                                zt[q], wt[:, 3 * cg + kw],
                                xv[:, cg, q, :, kw:kw + W],
                                start=(cg == 0 and kw == 0), stop=False)

                for q in range(NPAIR):
                    for kw in range(3):
                        nc.tensor.matmul(
                            zt[q], wt[:, 6 + kw],
                            xv[:, 2, q, :, kw:kw + W],
                            start=False, stop=(kw == 2))
                    # per-pair PSUM deinterleave (32p forced by PSUM gate
                    # groups); partitions g*32+r -> planar q*32+r, bf16 out.
                    # PE already applied 0.2*z + bias via the constant row.
                    sl = slice(q * RT, (q + 1) * RT)
                    zi, zf, zc, zo = (zt[q][g_ * RT:(g_ + 1) * RT]
                                      for g_ in range(4))
                    nc.scalar.activation(out=sc[sl], in_=zc, func=AF.Sigmoid,
                                         bias=0.0, scale=1.0)
                    nc.scalar.activation(out=go[sl], in_=zo, func=AF.Relu,
                                         bias=0.0, scale=1.0)
                    nc.vector.tensor_scalar(
                        out=gi[sl], in0=zi, scalar1=0.0, scalar2=1.0,
                        op0=OP.max, op1=OP.min)
                    nc.vector.tensor_scalar(
                        out=gf[sl], in0=zf, scalar1=0.0, scalar2=1.0,
                        op0=OP.max, op1=OP.min)

                # --- full-width (128p) bf16 gate math ---
                nc.vector.tensor_scalar(out=go, in0=go, scalar1=1.0,
                                        scalar2=0.0, op0=OP.min, op1=OP.max)
                nc.vector.tensor_tensor(out=t2, in0=gf, in1=cbuf, op=OP.mult)
                nc.vector.tensor_tensor(out=gi, in0=gi, in1=sc, op=OP.mult)
                nc.vector.tensor_tensor(out=cbuf, in0=gi, in1=t2, op=OP.add)
                nc.scalar.activation(out=s2, in_=cbuf, func=AF.Sigmoid,
                                     bias=0.0, scale=1.0)

                if t + 1 < Tn:
                    nc.vector.tensor_tensor(out=hbuf[:, :, 1:W + 1], in0=go,
                                            in1=s2, op=OP.mult)
                    # h scatter into next step's halo windows (engine ops
                    # can only shift partitions by 32s -> DMA).  tau and
                    # tau+4 share one planar partition range and are
                    # adjacent in tau_hat: 1 DMA per pair (blk <-> tau_hat
                    # lsb), + 2 single-row leftovers.
                    for dlo, dhi, p0, q in WPAIRS:
                        nc.sync.dma_start(
                            out=xt[dlo:dhi, t + 1, 2, 2 * q:2 * q + 2, :],
                            in_=hbuf[p0:p0 + (dhi - dlo), :, :])
                        if q == 0:  # tau4 (th 1) row 127 = blk0 p127
                            nc.sync.dma_start(
                                out=xt[68:69, t + 1, 2, 1, :],
                                in_=hbuf[127:128, 0, :])
                    # tau3 (th 6) row 128 = blk1 p0
                    nc.sync.dma_start(out=xt[101:102, t + 1, 2, 6, :],
                                      in_=hbuf[0:1, 1, :])
                    if t + 2 < Tn:
                        load_x(t + 2)
                else:
                    # final step: h = go*s2 per pair -> DRAM
                    hf = gpool.tile([128, 2, W], F32, tag="hf", name="hf")
                    ov = out_d.rearrange("(b p) w -> p b w", p=128)
                    oeng = (nc.sync, nc.gpsimd)
                    for q in range(NPAIR):
                        sl = slice(q * RT, (q + 1) * RT)
                        nc.vector.tensor_tensor(
                            out=hf[sl], in0=go[sl], in1=s2[sl], op=OP.mult)
                        oeng[q % 2].dma_start(out=ov[sl], in_=hf[sl])
    nc.compile()
    return nc


_CACHE = {}


def _get_program(Tn):
    if Tn not in _CACHE:
        _CACHE[Tn] = build_program(Tn)
    return _CACHE[Tn]


def kernel(x, Wx, Wh, b, _run_opts=None):
    x = np.asarray(x, dtype=np.float32)
    Bn, Tn = x.shape[0], x.shape[1]
    xk, wb = pack_inputs(x, Wx, Wh, b)
    nc = _get_program(Tn)
    zd = np.zeros((2, Tn, WP), dtype=ml_dtypes.bfloat16)
    in_maps = [{"xk": np.ascontiguousarray(xk[bi]), "wb": wb, "zd": zd}
               for bi in range(Bn)]
    res = bass_utils.run_bass_kernel_spmd(
        nc, in_maps, core_ids=list(range(Bn)), **(_run_opts or {}))
    out = np.stack([res.results[bi]["out"] for bi in range(Bn)], axis=0)
    kernel.last_results = res
    return out[..., None].astype(np.float32)


# revision 21
# speedup vs baseline: 1.5339x; 1.1776x over previous
"""ConvLSTM2D forward on 8 Trainium2 NeuronCores (v4.6).

Problem: x [8,10,256,256,8], Wx [3,3,8,4], Wh [3,3,1,4], b [4]
         -> h_last [8,256,256,1]  (ConvLSTM, keras gate order i,f,c,o;
         i/f/o hard_sigmoid, candidate+output sigmoid)

Sharding: data-parallel over batch; core b computes batch element b fully
locally (recurrent scan stays on-core, no collectives in forward).

v4.6 (baseline v3 was 213us).  Measured platform facts this build is
shaped around: SDMA engines are bound to fixed partition bands, so DMA
bandwidth = (engines covered) x ~18GB/s; the SWDGE (gpsimd) queue
generates only ~9 descriptors/us; sync/scalar HWDGE issue instructions
cost ~0.7-1.2us of engine time each; GpSimd ALU is ~6x slower than DVE.

 - h lives at partitions 0-33 (slot cc0 of channel-group 2, whose three
   matmul passes run last) so the per-step h window scatter shares the
   8-engine LOW partition band instead of oversubscribing the high one.
 - x channels fill the remaining 8 slots; per-step loads split by
   partition band and generation budget: gpsimd [34:88] (54 descs),
   sync [0:34]x(cg0,cg1) + [88:103], scalar the two single-row windows.
 - tau stored pair-major (tau_hat=2q+b): each window pair (tau,tau+4)
   shares one planar partition range -> 4 pair DMAs + 2 singles.
 - x in 3 rotating SBUF slots: load(t) overwrites slot t%3 giving the
   scheduler a natural 3-step prefetch bound (no ring flood).
 - gate affine folded INTO the matmul (0.2-scaled weights, K=103 with a
   DRAM constant-1 bias row applied once on the first pass).
 - all-bf16 epilogue; PSUM deinterleave 32p ops on DVE/ACT only.
"""

import numpy as np
import ml_dtypes

import concourse.bacc as bacc
import concourse.bass as bass
import concourse.mybir as mybir
import concourse.tile as tile
from concourse import bass_utils

F32 = mybir.dt.float32
BF16 = mybir.dt.bfloat16
AF = mybir.ActivationFunctionType
OP = mybir.AluOpType

B, T, H, W, CIN = 8, 10, 256, 256, 8
G = 4            # gates i,f,c,o
RT = 32          # output rows per tile (M = G*RT = 128)
TAU = H // RT    # 8 row tiles
HIN = RT + 2     # input rows per tile (with halo)
CPG = 3          # channels per contraction group
NCG = 3          # channel groups
KP = HIN * CPG   # 102 partitions of conv data per rhs tile
KB = KP + 1      # +1 constant-1 bias row
NPAIR = TAU // 2 # 4 tau-pairs (tau=4b+q at tau_hat=2q+b) -> N=512 matmuls
WP = W + 2       # padded width
NSLOT = 3        # rotating x slots in SBUF

# channel -> (cg, cc) slot map; (2, 0) is h (partitions 0-33)
CHSLOT = [(0, 0), (1, 0), (0, 1), (1, 1), (2, 1), (0, 2), (1, 2), (2, 2)]
# matmul pass order: h's group (2) last
PASSES = [(0, 0), (0, 1), (0, 2), (1, 0), (1, 1), (1, 2)]
HPASSES = [(2, 0), (2, 1), (2, 2)]

# h-window pair DMAs: (dst_lo, dst_hi, src_p0, q); dst partition = s (h at
# cc0), src hbuf[p0 + (s - s_lo), blk], tau_hat (2q, 2q+1) <-> blk (0, 1)
WPAIRS = [(1, 34, 0, 0), (0, 34, 31, 1), (0, 34, 63, 2), (0, 33, 95, 3)]


def pack_inputs(x, Wx, Wh, b):
    """Host-side repack to bf16 device layouts (partition-major x,
    pair-major tau, h at slot cc0 of group 2)."""
    x = np.asarray(x, dtype=np.float32)
    b = np.asarray(b, dtype=np.float32)
    Wx = np.asarray(Wx, np.float32)
    Wh = np.asarray(Wh, np.float32)
    gscale = np.array([0.2, 0.2, 1.0, 0.2], np.float32)
    gbias = np.array([0.2 * b[0] + 0.5, 0.2 * b[1] + 0.5,
                      b[2], 0.2 * b[3] + 0.5], np.float32)

    xk = np.zeros((B, KB, T, NCG, TAU, WP), dtype=ml_dtypes.bfloat16)
    xb = x.astype(ml_dtypes.bfloat16)
    for tau in range(TAU):
        th = 2 * (tau % NPAIR) + tau // NPAIR  # pair-major storage index
        r0 = tau * RT - 1
        lo = max(0, -r0)
        hi = min(HIN, H - r0)
        for ch in range(CIN):
            cg, cc = CHSLOT[ch]
            xk[:, cc * HIN + lo:cc * HIN + hi, :, cg, th, 1:W + 1] = \
                np.moveaxis(xb[:, :, r0 + lo:r0 + hi, :, ch], 1, 2)
    xk[:, KP, :, 0] = 1.0  # bias row (read by the first pass (0,0))

    wb = np.zeros((KB, NCG * 3, G * RT), dtype=np.float32)
    r = np.arange(RT)
    for kh in range(3):
        for kw in range(3):
            for g in range(G):
                for ch in range(CIN):
                    cg, cc = CHSLOT[ch]
                    wb[cc * HIN + r + kh, 3 * cg + kw, g * RT + r] = \
                        Wx[kh, kw, ch, g] * gscale[g]
                # h: group 2, cc 0
                wb[r + kh, 6 + kw, g * RT + r] = Wh[kh, kw, 0, g] * gscale[g]
    for g in range(G):
        wb[KP, 0, g * RT + r] = gbias[g]
    return xk, wb.astype(ml_dtypes.bfloat16)


def build_program(Tn):
    nc = bacc.Bacc("TRN2", target_bir_lowering=False, debug=False)
    xk_d = nc.dram_tensor("xk", [KB, Tn, NCG, TAU, WP], BF16, kind="ExternalInput")
    zd_d = nc.dram_tensor("zd", [2, NSLOT, WP], BF16, kind="ExternalInput")
    wb_d = nc.dram_tensor("wb", [KB, NCG * 3, G * RT], BF16, kind="ExternalInput")
    out_d = nc.dram_tensor("out", [H, W], F32, kind="ExternalOutput")

    with tile.TileContext(nc) as tc:
        with tc.tile_pool(name="wpool", bufs=1) as wpool, \
             tc.tile_pool(name="gpool", bufs=2) as gpool, \
             tc.tile_pool(name="state", bufs=1) as state, \
             tc.tile_pool(name="zpsum", bufs=2, space="PSUM") as zpsum:

            # --- static weights (split across both HWDGE rings) / x slots ---
            wt = wpool.tile([KB, NCG * 3, G * RT], BF16, tag="wt", name="wt")
            nc.sync.dma_start(out=wt[0:64], in_=wb_d[0:64])
            nc.scalar.dma_start(out=wt[64:KB], in_=wb_d[64:KB])

            xt = state.tile([KB, NSLOT, NCG, TAU, WP], BF16, tag="xt",
                            name="xt")

            def load_x(t):
                s = t % NSLOT
                nc.gpsimd.dma_start(out=xt[34:88, s], in_=xk_d[34:88, t])
                if t == 0:
                    nc.sync.dma_start(out=xt[0:34, 0], in_=xk_d[0:34, 0])
                else:
                    # skip cg2 = the h section (windows write it; no WAW)
                    nc.sync.dma_start(out=xt[0:34, s, 0:2],
                                      in_=xk_d[0:34, t, 0:2])
                nc.sync.dma_start(out=xt[88:KB, s], in_=xk_d[88:KB, t])

            load_x(0)
            if Tn > 1:
                # halo edge rows of the h sections (never window-written):
                # tau0 row -1 (p0, th0) and tau7 row 256 (p33, th7)
                nc.scalar.dma_start(out=xt[0:1, 1:NSLOT, 2, 0],
                                    in_=zd_d[0, 1:NSLOT])
                nc.scalar.dma_start(out=xt[33:34, 1:NSLOT, 2, TAU - 1],
                                    in_=zd_d[1, 1:NSLOT])
                load_x(1)

            cbuf = state.tile([128, 2, W], BF16, tag="cbuf", name="cbuf")
            nc.vector.memset(cbuf, 0.0)
            hbuf = state.tile([128, 2, WP], BF16, tag="hbuf", name="hbuf")
            nc.vector.memset(hbuf, 0.0)

            for t in range(Tn):
                # --- matmuls: 9 accumulating passes x 4 tau-pairs, N=512 ---
                xv = xt[:, t % NSLOT].rearrange("p cg (q b) c -> p cg q b c",
                                                q=NPAIR)
                zt = [zpsum.tile([G * RT, 2, W], F32, tag=f"z{q}", name=f"z{q}")
                      for q in range(NPAIR)]
                gi = gpool.tile([128, 2, W], BF16, tag="gi", name="gi")
                gf = gpool.tile([128, 2, W], BF16, tag="gf", name="gf")
                go = gpool.tile([128, 2, W], BF16, tag="go", name="go")
                sc = gpool.tile([128, 2, W], BF16, tag="sc", name="sc")
                s2 = gpool.tile([128, 2, W], BF16, tag="s2", name="s2")
                t2 = gpool.tile([128, 2, W], BF16, tag="t2", name="t2")

                for pi, (cg, kw) in enumerate(PASSES):
                    for q in range(NPAIR):
                        nc.tensor.matmul(
                            zt[q], wt[:, 3 * cg + kw],
                            xv[:, cg, q, :, kw:kw + W],
                            start=(pi == 0), stop=False)

                for q in range(NPAIR):
                    for pi, (cg, kw) in enumerate(HPASSES):
                        nc.tensor.matmul(
                            zt[q], wt[:, 3 * cg + kw],
                            xv[:, cg, q, :, kw:kw + W],
                            start=False, stop=(pi == 2))
                    # per-pair PSUM deinterleave (32p forced by PSUM gate
                    # groups); partitions g*32+r -> planar q*32+r, bf16 out.
                    # PE already applied 0.2*z + bias via the constant row.
                    sl = slice(q * RT, (q + 1) * RT)
                    zi, zf, zc, zo = (zt[q][g_ * RT:(g_ + 1) * RT]
                                      for g_ in range(4))
                    nc.scalar.activation(out=sc[sl], in_=zc, func=AF.Sigmoid,
                                         bias=0.0, scale=1.0)
                    nc.scalar.activation(out=go[sl], in_=zo, func=AF.Relu,
                                         bias=0.0, scale=1.0)
                    nc.vector.tensor_scalar(
                        out=gi[sl], in0=zi, scalar1=0.0, scalar2=1.0,
                        op0=OP.max, op1=OP.min)
                    nc.vector.tensor_scalar(
                        out=gf[sl], in0=zf, scalar1=0.0, scalar2=1.0,
                        op0=OP.max, op1=OP.min)

                # --- full-width (128p) bf16 gate math ---
                nc.vector.tensor_scalar(out=go, in0=go, scalar1=1.0,
                                        scalar2=0.0, op0=OP.min, op1=OP.max)
                nc.vector.tensor_tensor(out=t2, in0=gf, in1=cbuf, op=OP.mult)
                nc.vector.tensor_tensor(out=gi, in0=gi, in1=sc, op=OP.mult)
                nc.vector.tensor_tensor(out=cbuf, in0=gi, in1=t2, op=OP.add)
                nc.scalar.activation(out=s2, in_=cbuf, func=AF.Sigmoid,
                                     bias=0.0, scale=1.0)

                if t + 1 < Tn:
                    nc.vector.tensor_tensor(out=hbuf[:, :, 1:W + 1], in0=go,
                                            in1=s2, op=OP.mult)
                    ns = (t + 1) % NSLOT
                    # h scatter into next slot's halo windows: tau and tau+4
                    # share one planar partition range and are adjacent in
                    # tau_hat -> 1 DMA per pair + 2 single-row leftovers.
                    for dlo, dhi, p0, q in WPAIRS:
                        nc.sync.dma_start(
                            out=xt[dlo:dhi, ns, 2, 2 * q:2 * q + 2, :],
                            in_=hbuf[p0:p0 + (dhi - dlo), :, :])
                    # tau4 (th1) row 127 = blk0 p127; tau3 (th6) row 128
                    nc.scalar.dma_start(out=xt[0:1, ns, 2, 1, :],
                                        in_=hbuf[127:128, 0, :])
                    nc.scalar.dma_start(out=xt[33:34, ns, 2, 6, :],
                                        in_=hbuf[0:1, 1, :])
                    if t + 2 < Tn:
                        load_x(t + 2)
                else:
                    # final step: h = go*s2 per pair -> DRAM
                    hf = gpool.tile([128, 2, W], F32, tag="hf", name="hf")
                    ov = out_d.rearrange("(b p) w -> p b w", p=128)
                    oeng = (nc.sync, nc.gpsimd)
                    for q in range(NPAIR):
                        sl = slice(q * RT, (q + 1) * RT)
                        nc.vector.tensor_tensor(
                            out=hf[sl], in0=go[sl], in1=s2[sl], op=OP.mult)
                        oeng[q % 2].dma_start(out=ov[sl], in_=hf[sl])
    nc.compile()
    return nc


_CACHE = {}


def _get_program(Tn):
    if Tn not in _CACHE:
        _CACHE[Tn] = build_program(Tn)
    return _CACHE[Tn]


def kernel(x, Wx, Wh, b, _run_opts=None):
    x = np.asarray(x, dtype=np.float32)
    Bn, Tn = x.shape[0], x.shape[1]
    xk, wb = pack_inputs(x, Wx, Wh, b)
    nc = _get_program(Tn)
    zd = np.zeros((2, NSLOT, WP), dtype=ml_dtypes.bfloat16)
    in_maps = [{"xk": np.ascontiguousarray(xk[bi]), "wb": wb, "zd": zd}
               for bi in range(Bn)]
    res = bass_utils.run_bass_kernel_spmd(
        nc, in_maps, core_ids=list(range(Bn)), **(_run_opts or {}))
    out = np.stack([res.results[bi]["out"] for bi in range(Bn)], axis=0)
    kernel.last_results = res
    return out[..., None].astype(np.float32)
